# revision 51
# baseline (speedup 1.0000x reference)
"""MemNN layer kernel for 8 Trainium2 NeuronCores.

Strategy (batch-sharded, 16 batches/core):
- The 4 embedding tables are interleaved into one fp16 "megatable" whose row v
  is [A0|A1|A2|A3] (4 x 128 fp16 = 1024 B). One dma_gather fetches all 4
  tables for a token; GpSimd descriptor-gen cost is per-index, not per-byte.
- Desc-gen is the gather bottleneck and is serialized per SWDGE queue, so the
  (b, s) slots are split across TWO queues whose desc-gen runs in parallel:
  * path A (slots 0..399 + query): transpose-mode gathers on queue 0 land
    embeddings embd-on-partition; 20-token sentence sums via DVE
    tensor_reduce over a 5-D AP -> S[:, :, :400] f32.
  * path B (slots 400..799): non-transpose gathers on queue 1 (tokens on
    partitions) in 128-token groups of 6 sentences (pad rows -> zero row 0);
    sentence sums on PE: out[embd, slot] += G[tok, embd-slice]^T @ SEL.
    Only one transpose-mode DMA stream ever exists (concurrent transpose
    streams from two queues corrupt on HW; plain DMA coexists fine).
- Hops in embd-on-partition layout: logits via elementwise mul + ones-matmul
  partition reduce, exp straight off PSUM (no max-subtract; |logit| <= ~40),
  p normalized to fp16, broadcast via e0-selector fp16 matmul, weighted c-sum
  via DVE reduce.
- Final projection out[v, b] = sum_e A3[v, e] u[e, b]: full A3^T fp16 is
  prefetched to SBUF during the gather phase (DMA engines are desc-gen
  starved there), so the projection is purely PE-bound; per-block
  PSUM -> SBUF -> DRAM stores overlap later matmuls.
"""

import numpy as np

HOPS = 3
VOCAB = 50000
EMBD = 128
BS = 128
STORY = 50
SENT = 20
QLEN = 20
NCORES = 8
BSH = BS // NCORES          # 16 batches per core
SLOTS = BSH * STORY         # 800 (b, s) slots per core
NROWS = VOCAB + 1           # 50001 (row 50000 spare zero row)
QPAD = 24                   # per-batch query tokens padded 20 -> 24
QIDX = BSH * QPAD           # 384 (%128 == 0)
VPAD = 50048                # vocab padded to 391*128 for projection
NVC = VPAD // 128           # 391 projection chunks

SLOTS_A = 400               # path A slots (transpose/DVE)
SLOTS_B = SLOTS - SLOTS_A   # 400 path B slots (non-transpose/PE)
GRP_B = 6                   # sentences per 128-token path-B group
NG_B = (SLOTS_B + GRP_B - 1) // GRP_B        # 67 groups
NIDX_B = NG_B * 128                           # 8576 gather idxs
CH_A = [(1280, 1280)] * 5 + [(640, 640)] * 2 + [(640, 320)]  # (gather_n, reduce_tokens); small tail chunks keep the reduce tail short
NIDX_A = sum(g for g, _ in CH_A)              # 8320 (320 pad idxs -> row 0)
CH_B = [1280] * 6 + [896]                     # 10 groups per chunk, last 7
GBUFS = 2

_cache = {}


def _wrap_idxs(lst):
    """int16 gather index layout: [128, n/16]; position i -> [i%16, i//16], tiled 8x."""
    a = np.asarray(lst).astype(np.int16).reshape(-1, 16).T.copy()
    return np.tile(a, (8, 1))


def _mk_ap(base_ap, dims, extra_offset_elems=0):
    """AP with the partition pair of base_ap and given free (stride, count) pairs."""
    import concourse.bass as bass
    ap = [tuple(base_ap.ap[0])] + [tuple(d) for d in dims]
    return bass.AP(base_ap.tensor, base_ap.offset + extra_offset_elems, ap)


def _build():
    import concourse.tile as tile
    from concourse import bacc, mybir

    f32 = mybir.dt.float32
    f16 = mybir.dt.float16
    i16 = mybir.dt.int16

    nc = bacc.Bacc("TRN2", target_bir_lowering=False, debug=False,
                   num_swdge_queues=2)

    mega = nc.dram_tensor("mega", [NROWS, 512], f16, kind="ExternalInput")
    a3t = nc.dram_tensor("a3t", [128, VPAD], f16, kind="ExternalInput")
    ila = nc.dram_tensor("ila", [128, NIDX_A // 16], i16, kind="ExternalInput")
    ilb = nc.dram_tensor("ilb", [128, NIDX_B // 16], i16, kind="ExternalInput")
    iqlo = nc.dram_tensor("iqlo", [128, QIDX // 16], i16, kind="ExternalInput")
    selt = nc.dram_tensor("selt", [128, GRP_B], f16, kind="ExternalInput")
    tat = nc.dram_tensor("tat", [128, STORY], f32, kind="ExternalInput")
    tct = nc.dram_tensor("tct", [128, STORY], f32, kind="ExternalInput")
    out = nc.dram_tensor("outp", [128, NVC * BSH], f32, kind="ExternalOutput")

    with tile.TileContext(nc) as tc:
        with (
            tc.tile_pool(name="consts", bufs=1) as cpool,
            tc.tile_pool(name="sacc", bufs=1) as spool,
        ):
            # ---- index loads first: they gate the first gathers
            t_iqlo = cpool.tile([128, QIDX // 16], i16, tag="iqlo")
            nc.sync.dma_start(t_iqlo[:], iqlo[:])
            t_ila = cpool.tile([128, NIDX_A // 16], i16, tag="ila")
            nc.sync.dma_start(t_ila[:], ila[:])
            t_ilb = cpool.tile([128, NIDX_B // 16], i16, tag="ilb")
            nc.sync.dma_start(t_ilb[:], ilb[:])

            t_tat = cpool.tile([128, STORY], f32, tag="tat")
            nc.sync.dma_start(t_tat[:], tat[:])
            t_tct = cpool.tile([128, STORY], f32, tag="tct")
            nc.sync.dma_start(t_tct[:], tct[:])
            ones_col = cpool.tile([128, 1], f32, tag="ones_col")  # lhsT for partition sum
            nc.vector.memset(ones_col[:], 1.0)
            e0row = cpool.tile([128, 128], f16, tag="e0row")      # lhsT for p broadcast
            nc.vector.memset(e0row[:], 0.0)
            nc.vector.memset(e0row[0:1, :], 1.0)
            sel = cpool.tile([128, GRP_B], f16, tag="sel")        # path-B sentence selector
            nc.sync.dma_start(sel[:], selt[:])

            # ---- S accumulator [128, 4 tables, 800 slots] f32
            S = spool.tile([128, 4, SLOTS], f32, tag="S")
            uq = spool.tile([128, 4, BSH], f32, tag="uq")  # query-sum per table

            with (
                tc.tile_pool(name="gath", bufs=3) as gpool,
                tc.tile_pool(name="gathb", bufs=3) as gpoolb,
                tc.tile_pool(name="bpsum", bufs=4, space="PSUM") as bpool,
            ):
                # query gather + reduce (queue 0, transpose)
                gq = gpool.tile([128, 4, QIDX], f16, tag="gq")
                nc.gpsimd.dma_gather(
                    gq[:], mega[:], t_iqlo[:], QIDX, QIDX, 512,
                    transpose=True, single_packet=False, queue_num=0)
                nc.vector.tensor_reduce(
                    uq[:], _mk_ap(gq[:], [(QIDX, 4), (QPAD, BSH), (1, QPAD)]),
                    mybir.AxisListType.X, mybir.AluOpType.add)

                posAg = posAr = 0
                gB = 0
                for i in range(max(len(CH_A), len(CH_B))):
                    if i < len(CH_A):
                        gn, rt = CH_A[i]
                        ga = gpool.tile([128, 4, gn], f16,
                                        tag="ga" if gn == CH_A[0][0] else "ga_s")
                        nc.gpsimd.dma_gather(
                            ga[:], mega[:],
                            t_ila[:, posAg // 16:(posAg + gn) // 16],
                            gn, gn, 512,
                            transpose=True, single_packet=False, queue_num=0)
                        nslot = rt // SENT
                        s0 = posAr // SENT
                        nc.vector.tensor_reduce(
                            S[:, :, s0:s0 + nslot],
                            _mk_ap(ga[:], [(gn, 4), (SENT, nslot), (1, SENT)]),
                            mybir.AxisListType.X, mybir.AluOpType.add)
                        posAg += gn
                        posAr += rt
                    if i < len(CH_B):
                        bn = CH_B[i]
                        ngrp = bn // 128
                        gb = gpoolb.tile([128, ngrp, 512], f16,
                                         tag="gb" if bn == CH_B[0] else "gb_s")
                        nc.gpsimd.dma_gather(
                            gb[:], mega[:],
                            t_ilb[:, gB * 8:(gB * 8 + bn // 16)],
                            bn, bn, 512,
                            transpose=False, single_packet=False, queue_num=1)
                        for g in range(ngrp):
                            sg = SLOTS_A + (gB + g) * GRP_B
                            ns = min(GRP_B, SLOTS - sg)
                            po = bpool.tile([128, 4 * GRP_B], f32, tag="pob",
                                            space="PSUM")
                            for k in range(4):
                                nc.tensor.matmul(
                                    po[:, k * GRP_B:k * GRP_B + GRP_B],
                                    lhsT=gb[:, g, k * 128:(k + 1) * 128],
                                    rhs=sel[:], start=True, stop=True)
                            # Scalar engine does the PSUM -> S copies so the
                            # in-order DVE stream (A reduces) never backs up
                            # path B's PSUM rotation.
                            nc.scalar.activation(
                                _mk_ap(S[:], [(SLOTS, 4), (1, ns)], sg),
                                _mk_ap(po[:], [(GRP_B, 4), (1, ns)]),
                                mybir.ActivationFunctionType.Copy)
                        gB += ngrp

            with (
                tc.tile_pool(name="hopp", bufs=1) as hpool,
                tc.tile_pool(name="psum", bufs=2, space="PSUM") as ppool,
            ):
                # u0 = query-sum of table 0
                u = hpool.tile([128, BSH], f32, tag="u")
                nc.vector.tensor_copy(u[:], uq[:, 0, :])

                t0 = hpool.tile([128, BSH, STORY], f32, tag="t0")
                t1 = hpool.tile([128, BSH, STORY], f32, tag="t1")
                pe_sb = hpool.tile([128, BSH, STORY], f16, tag="pe_sb")
                nc.vector.memset(pe_sb[:], 0.0)
                lg = hpool.tile([1, BSH, STORY], f32, tag="lg")
                red2 = hpool.tile([1, BSH], f32, tag="red2")
                red_u = hpool.tile([128, BSH], f32, tag="redu")

                def smv(k, off=0, nb=BSH):
                    return _mk_ap(S[:], [(STORY, nb), (1, STORY)], k * SLOTS + off * STORY)

                def t0v(off=0, nb=BSH):
                    return _mk_ap(t0[:], [(STORY, nb), (1, STORY)], off * STORY)

                def t1v(off=0, nb=BSH):
                    return _mk_ap(t1[:], [(STORY, nb), (1, STORY)], off * STORY)

                def t0f(off, n):
                    return _mk_ap(t0[:], [(1, n)], off)

                ta_b = _mk_ap(t_tat[:], [(0, BSH), (1, STORY)])
                tc_bh = _mk_ap(t_tct[:], [(0, BSH // 2), (1, STORY)])
                u_b = _mk_ap(u[:], [(1, BSH), (0, STORY)])
                HB = SLOTS // 2  # 400

                ta_bh = _mk_ap(t_tat[:], [(0, BSH // 2), (1, STORY)])
                for k in range(HOPS):
                    # t0 = (S[k] + TA bcast) * u bcast, in halves so the PE
                    # logit reduce of half 0 overlaps DVE work on half 1.
                    # exp straight off PSUM. No max-subtract: |logit| <= ~40
                    # for this model scale, exp stays inside f32.
                    for h in range(2):
                        hb = h * (BSH // 2)
                        u_bh = _mk_ap(u[:], [(1, BSH // 2), (0, STORY)], hb)
                        nc.vector.tensor_add(
                            t0v(hb, BSH // 2), smv(k, hb, BSH // 2), ta_bh)
                        nc.vector.tensor_mul(
                            t0v(hb, BSH // 2), t0v(hb, BSH // 2), u_bh)
                        pl = ppool.tile([1, HB], f32, tag="pl", space="PSUM")
                        nc.tensor.matmul(
                            pl[:], lhsT=ones_col[:], rhs=t0f(h * HB, HB),
                            start=True, stop=True)
                        nc.scalar.activation(
                            _mk_ap(lg[:], [(1, HB)], h * HB), pl[:],
                            mybir.ActivationFunctionType.Exp)
                    # c-side t1 = S[k+1] + TC does not depend on the softmax:
                    # emit it here so DVE does it under the PE/Act exp latency.
                    for h in range(2):
                        nc.vector.tensor_add(
                            t1v(h * (BSH // 2), BSH // 2),
                            smv(k + 1, h * (BSH // 2), BSH // 2), tc_bh)
                    nc.vector.tensor_reduce(red2[:], lg[:], mybir.AxisListType.X, mybir.AluOpType.add)
                    nc.vector.reciprocal(red2[:], red2[:])
                    red2_b = _mk_ap(red2[:], [(1, BSH), (0, STORY)])
                    nc.vector.tensor_mul(pe_sb[0:1, :, :], lg[:], red2_b)
                    # broadcast p to all partitions; then t0 = (S[k+1] + TC bcast) * p
                    for h in range(2):
                        pb = ppool.tile([128, HB], f32, tag="pb", space="PSUM")
                        nc.tensor.matmul(
                            pb[:], lhsT=e0row[:],
                            rhs=_mk_ap(pe_sb[:], [(1, HB)], h * HB),
                            start=True, stop=True)
                        pb3 = _mk_ap(pb[:], [(STORY, BSH // 2), (1, STORY)])
                        nc.vector.tensor_mul(
                            t1v(h * (BSH // 2), BSH // 2),
                            t1v(h * (BSH // 2), BSH // 2), pb3)
                    # u += sum_s p*c
                    nc.vector.tensor_reduce(red_u[:], t1v(), mybir.AxisListType.X, mybir.AluOpType.add)
                    nc.vector.tensor_add(u[:], u[:], red_u[:])

                # ---- projection: out[v, b] = sum_e A3[v, e] * u[e, b]
                # Per-block PSUM -> SBUF -> DRAM so stores overlap later matmuls.
                u16 = hpool.tile([128, BSH], f16, tag="u16")
                nc.vector.tensor_copy(u16[:], u[:])
                with (
                    tc.tile_pool(name="a3pool", bufs=13) as apool,
                    tc.tile_pool(name="opool", bufs=3) as opool,
                ):
                    CPL = 32  # vocab chunks (of 128) per block; 32*16 = 512 f32 = 1 PSUM bank
                    for blk in range(NVC // CPL + (1 if NVC % CPL else 0)):
                        n_in_blk = min(CPL, NVC - blk * CPL)
                        b0 = blk * CPL * 128
                        a3c = apool.tile([128, CPL * 128], f16, tag="a3c")
                        nc.sync.dma_start(
                            a3c[:, :n_in_blk * 128],
                            a3t[:, b0:b0 + n_in_blk * 128])
                        po = ppool.tile([128, CPL * BSH], f32, tag="po", space="PSUM")
                        for w in range(n_in_blk):
                            nc.tensor.matmul(
                                po[:, w * BSH:(w + 1) * BSH],
                                lhsT=a3c[:, w * 128:(w + 1) * 128],
                                rhs=u16[:], start=True, stop=True)
                        ob = opool.tile([128, CPL * BSH], f32, tag="ob")
                        nc.vector.tensor_copy(
                            ob[:, :n_in_blk * BSH], po[:, :n_in_blk * BSH])
                        c0 = blk * CPL * BSH
                        nc.sync.dma_start(
                            out[:, c0:c0 + n_in_blk * BSH],
                            ob[:, :n_in_blk * BSH])

    nc.compile()
    return nc


def _prep_inputs(x, q, A, TA, TC):
    """Host-side marshalling: fp16 megatable, A3^T, per-core index lists."""
    x = np.asarray(x).astype(np.int64)
    q = np.asarray(q).astype(np.int64)
    A = np.asarray(A, dtype=np.float32)
    TA = np.asarray(TA, dtype=np.float32)
    TC = np.asarray(TC, dtype=np.float32)

    mega = np.zeros((NROWS, 512), dtype=np.float16)
    for k in range(4):
        mega[:VOCAB, k * 128:(k + 1) * 128] = A[k].astype(np.float16)

    a3t = np.zeros((128, VPAD), dtype=np.float16)
    a3t[:, :VOCAB] = A[3].astype(np.float16).T

    tat = np.ascontiguousarray(TA[0].T)  # [128, 50]
    tct = np.ascontiguousarray(TC[0].T)
    selt = np.zeros((128, GRP_B), dtype=np.float16)
    for s in range(GRP_B):
        selt[s * SENT:(s + 1) * SENT, s] = 1.0

    in_maps = []
    for c in range(NCORES):
        xs = x[c * BSH:(c + 1) * BSH].reshape(SLOTS, SENT)  # [800, 20]
        qs = q[c * BSH:(c + 1) * BSH].reshape(-1)           # [320]
        # Per-core row permutation: rows this core touches (plus the zero
        # padding row 0) move to the front so every remapped index fits int16.
        used = np.unique(np.concatenate([[0], xs.reshape(-1), qs]))
        nu = used.shape[0]
        order = np.empty(NROWS, dtype=np.int64)
        order[:nu] = used
        mask = np.ones(NROWS, dtype=bool)
        mask[used] = False
        order[nu:] = np.nonzero(mask)[0]
        pos = np.empty(NROWS, dtype=np.int64)
        pos[order] = np.arange(NROWS)
        mega_c = mega[order]
        idx = pos[xs]                                        # [800, 20]
        # path A: slots 0..SLOTS_A-1, flat + pad to NIDX_A with row 0
        ia = np.zeros(NIDX_A, dtype=np.int64)
        ia[:SLOTS_A * SENT] = idx[:SLOTS_A].reshape(-1)
        # path B: 128-token groups of GRP_B sentences (pad -> row 0)
        ib = np.zeros((NG_B, 128), dtype=np.int64)
        sl = idx[SLOTS_A:]
        for g in range(NG_B):
            s0 = g * GRP_B
            ns = min(GRP_B, SLOTS_B - s0)
            ib[g, :ns * SENT] = sl[s0:s0 + ns].reshape(-1)
        qp = np.zeros(BSH * QPAD, dtype=np.int64)            # pad -> row 0 (zeros)
        qp2 = qp.reshape(BSH, QPAD)
        qp2[:, :QLEN] = pos[qs].reshape(BSH, QLEN)
        in_maps.append({
            "mega": mega_c, "a3t": a3t, "tat": tat, "tct": tct, "selt": selt,
            "ila": _wrap_idxs(ia), "ilb": _wrap_idxs(ib.reshape(-1)),
            "iqlo": _wrap_idxs(qp),
        })
    return in_maps, 1.0


def kernel(x, q, A, TA, TC):
    import os
    from concourse.bass_utils import run_bass_kernel_spmd

    in_maps, _ = _prep_inputs(x, q, A, TA, TC)
    if "nc" not in _cache:
        _cache["nc"] = _build()
    nc = _cache["nc"]
    trace = bool(int(os.environ.get("MEMNN_TRACE", "0")))
    res = run_bass_kernel_spmd(nc, in_maps, list(range(NCORES)), trace=trace)
    if trace:
        _cache["exec_time_ns"] = res.exec_time_ns
        _cache["mean_exec_time_ns"] = res.mean_exec_time_ns
        _cache["results"] = res

    outs = []
    for c in range(NCORES):
        oc = res.results[c]["outp"].reshape(128, NVC, BSH)
        full = oc.transpose(1, 0, 2).reshape(VPAD, BSH)   # [50048, 16]
        outs.append(full[:VOCAB].T)                       # [16, 50000]
    return np.concatenate(outs, axis=0).astype(np.float32)
